# revision 2
# baseline (speedup 1.0000x reference)
"""Continuous Wavelet Transform (4-scale Morlet, 129-tap) on 8 TRN2 NeuronCores.

The reference pads H and W by 3 and crops back after a conv along W — the
pad/crop cancels exactly, so the whole module reduces to a SAME 129-tap
correlation of each of the B*C*H rows with 4 wavelet kernels.

Strategy (data-parallel over B, one batch element per core):
  out[w] = sum_k ker[k] * x[w + k - 64]
With x zero-padded by 64 on each side (X, length 1152) and tiled in 128-wide
tiles XT_m, each 128-wide output tile j is exactly two matmuls:
  out_j[q] = sum_p XT_j[p] * P[p,q] + sum_p XT_{j+1}[p] * Q[p,q]
  P[p,q] = ker[p-q]     (p >= q, lower-triangular Toeplitz)
  Q[p,q] = ker[128+p-q] (p <= q, upper-triangular Toeplitz)

Multirate trick: the scale-8 / scale-16 responses are band-limited (Gaussian
spectrum at w0/s with sigma 1/s; content beyond pi/2 resp. pi/4 is ~1e-13 of
peak), so the device emits them decimated by 2x / 4x over the full padded
support [-64, 1088) and the host reconstructs the full rate exactly with an
FFT zero-pad (the 1152-long support makes the circular embedding exact).
This cuts both output HBM traffic and matmul columns by ~1/3.

Per output tile j the 352 live columns are permuted into
  [B-only 119 | shared 114 | A-only 119]
so the A matmul (x-tile j) writes the single contiguous PSUM range [119:352)
and the B matmul (x-tile j+1) writes [0:233). Decimated samples outside
[0,1024) come from two 48-col edge matmuls (tile -1 needs only XT_0, tile 8
only XT_8) accumulated into spare PSUM columns [352:400) of tiles j=0/j=7.
"""
import numpy as np
import ml_dtypes

import concourse.bacc as bacc
import concourse.mybir as mybir
import concourse.tile as tile
from concourse.bass_utils import run_bass_kernel_spmd

BF16 = ml_dtypes.bfloat16
N_CORES = 8
B, C, H, W = 8, 16, 128, 1024
SCALES = (2.0, 4.0, 8.0, 16.0)
MORLET_W0 = 5.0
ROWS = C * H              # 2048 rows per core
CHUNKS = ROWS // 128      # 16 row-chunks (chunk == channel)
JT = W // 128             # 8 main output W-tiles
MT = JT + 1               # 9 stationary x tiles
XLEN = 128 * MT           # 1152 = padded x length

COMPUTE_DT = mybir.dt.bfloat16
COMPUTE_NP = BF16

GROUPS = 8                     # row groups per core
GROUP_COLS = ROWS // GROUPS    # 256 rows per group
CHUNKS_PER_GROUP = GROUP_COLS // 128  # 2

HW = (8, 15, 36, 64)      # kept half-width per scale (tail beyond is ~<1e-3)
DEC = (1, 1, 2, 4)        # output decimation per scale (band-limited scales)
REPS_UNROLL = 4           # reps unrolled inside each For_i iteration


def _wavelet_bank():
    t = np.arange(-64, 65, dtype=np.float32)  # [129]
    return np.stack([
        np.exp(-0.5 * (t / s) ** 2) * np.cos(MORLET_W0 * t / s) / np.sqrt(s)
        for s in SCALES
    ]).astype(np.float32)  # [4, 129]


def _sections():
    """Per-scale q lists for the three main column sections of one j-tile.

    A (x-tile j, weights P) covers q < 64+h; B (x-tile j+1, weights Q)
    covers q >= 64-h; decimated scales only keep q % dec == 0."""
    bonly, shared, aonly = [], [], []
    for s, (h, d) in enumerate(zip(HW, DEC)):
        qs = np.arange(0, 128, d)
        bonly.append([(s, q) for q in qs if q >= 64 + h])
        shared.append([(s, q) for q in qs if 64 - h <= q < 64 + h])
        aonly.append([(s, q) for q in qs if q < 64 - h])
    bcols = [sq for sec in bonly for sq in sec]
    scols = [sq for sec in shared for sq in sec]
    acols = [sq for sec in aonly for sq in sec]
    return bcols, scols, acols


_BCOLS, _SCOLS, _ACOLS = _sections()
BW_, SW_, AW_ = len(_BCOLS), len(_SCOLS), len(_ACOLS)   # 119, 114, 119
MW = BW_ + SW_ + AW_                                     # 352 main cols/tile
WA_W = SW_ + AW_                                         # 233 (A writes [BW_:MW))
WB_W = BW_ + SW_                                         # 233 (B writes [0:WB_W))
EDGE_L = [(2, q) for q in range(64, 128, 2)] + [(3, q) for q in range(64, 128, 4)]
EDGE_R = [(2, q) for q in range(0, 64, 2)] + [(3, q) for q in range(0, 64, 4)]
EW = len(EDGE_L)                                         # 48
OUT_W = JT * MW + 2 * EW                                 # 2912
PS_W = MW + EW                                           # 400 (fits a PSUM bank)
WT_W = (WA_W + EW) + WB_W + EW                           # [WA|WAR... see _weights


def _toeplitz(bank, s, q, kind):
    """Column [128] of P (kind 'P': taps t=p-q-64) or Q (taps t=64+p-q)."""
    p = np.arange(128)
    t = (p - q - 64) if kind == "P" else (64 + p - q)
    col = np.zeros(128, np.float32)
    m = np.abs(t) <= HW[s]
    col[m] = bank[s][t[m] + 64]
    return col


def _weights():
    """Packed weight blob [128, 562] = [WA+WBL fused 281 | WB 233 | WAR 48].

    m=0 does one fused matmul: A cols for j=0 plus the 48 edge-L cols
    (psum [119:400) is contiguous), so WBL is packed right after WA."""
    bank = _wavelet_bank()
    wa = np.stack([_toeplitz(bank, s, q, "P") for s, q in _SCOLS + _ACOLS], 1)
    wbl = np.stack([_toeplitz(bank, s, q, "Q") for s, q in EDGE_L], 1)
    wb = np.stack([_toeplitz(bank, s, q, "Q") for s, q in _BCOLS + _SCOLS], 1)
    war = np.stack([_toeplitz(bank, s, q, "P") for s, q in EDGE_R], 1)
    wt = np.concatenate([wa, wbl, wb, war], axis=1)
    assert wt.shape == (128, WT_W), wt.shape
    return np.ascontiguousarray(wt.astype(COMPUTE_NP))


def _build_nc(reps=1):
    nc = bacc.Bacc("TRN2", target_bir_lowering=False, debug=False,
                   num_devices=N_CORES)
    # xt[g, p, m, c]: row-group, position-in-tile, x-tile, row-in-group —
    # per-partition contiguous so the input DMA needs no rearrange
    xt_d = nc.declare_dram_parameter("xt", [GROUPS, 128, MT, GROUP_COLS],
                                     COMPUTE_DT, isOutput=False)
    wt_d = nc.declare_dram_parameter("wt", [128, WT_W], COMPUTE_DT,
                                     isOutput=False)
    # out[r, h, col]: chunk-r (=channel), H, permuted col (8x352 main + edges)
    out_d = nc.declare_dram_parameter("out", [CHUNKS, 128, OUT_W],
                                      COMPUTE_DT, isOutput=True)

    f32 = mybir.dt.float32
    with tile.TileContext(nc) as tc:
        with (
            tc.tile_pool(name="consts", bufs=1) as consts,
            tc.tile_pool(name="xpool", bufs=5) as xpool,
            tc.tile_pool(name="opool", bufs=3) as opool,
            tc.tile_pool(name="psum", bufs=6, space="PSUM") as psum_pool,
            tc.tile_pool(name="warm", bufs=1, space="PSUM") as warm_pool,
        ):
            def chunk_body(r, lhs_of_m, last_chunk):
                outbuf = opool.tile([128, OUT_W], COMPUTE_DT,
                                    name="outbuf", tag="outbuf")
                ps = [None] * JT
                for m in range(MT):
                    lhs = lhs_of_m(m)
                    if m < JT:
                        ps[m] = psum_pool.tile([128, PS_W], f32,
                                               name="ps", tag="ps")
                        if m == 0:
                            # fused A(j=0) + edge-L: psum [119:400) contiguous
                            nc.tensor.matmul(ps[0][:, BW_:PS_W], lhs,
                                             wal[:], start=True, stop=False)
                        else:
                            nc.tensor.matmul(ps[m][:, BW_:MW], lhs,
                                             wa[:], start=True, stop=False)
                    if m >= 1:
                        j = m - 1
                        nc.tensor.matmul(ps[j][:, 0:WB_W], lhs, wb[:],
                                         start=False, stop=True)
                        if m == MT - 1:
                            nc.tensor.matmul(ps[j][:, MW:PS_W], lhs, war[:],
                                             start=True, stop=True)
                        dst = outbuf[:, j * MW:(j + 1) * MW]
                        if j % 2 == 0:
                            nc.scalar.copy(dst, ps[j][:, 0:MW])
                        else:
                            nc.vector.tensor_copy(dst, ps[j][:, 0:MW])
                        if j == 0:
                            nc.scalar.copy(outbuf[:, JT * MW:JT * MW + EW],
                                           ps[0][:, MW:PS_W])
                        elif j == JT - 1:
                            nc.vector.tensor_copy(outbuf[:, JT * MW + EW:OUT_W],
                                                  ps[j][:, MW:PS_W])
                        if last_chunk:
                            # quarter-granularity drain of the final chunk
                            if j in (1, 3, 5):
                                nc.sync.dma_start(
                                    out_d[r, :, (j - 1) * MW:(j + 1) * MW],
                                    outbuf[:, (j - 1) * MW:(j + 1) * MW])
                        elif j == 3:
                            nc.sync.dma_start(out_d[r, :, 0:4 * MW],
                                              outbuf[:, 0:4 * MW])
                if last_chunk:
                    nc.sync.dma_start(out_d[r, :, 6 * MW:OUT_W],
                                      outbuf[:, 6 * MW:OUT_W])
                else:
                    nc.sync.dma_start(out_d[r, :, 4 * MW:OUT_W],
                                      outbuf[:, 4 * MW:OUT_W])

            wal = consts.tile([128, WA_W + EW], COMPUTE_DT)  # [WA | WBL]
            wb = consts.tile([128, WB_W], COMPUTE_DT)
            war = consts.tile([128, EW], COMPUTE_DT)
            wa = wal[:, 0:WA_W]

            nc.sync.dma_start(wal[:], wt_d[:, 0:WA_W + EW])
            nc.sync.dma_start(wb[:], wt_d[:, WA_W + EW:WA_W + EW + WB_W])
            nc.sync.dma_start(war[:], wt_d[:, WA_W + EW + WB_W:WT_W])

            # Warm the PE clock gate during the input-DMA head: back-to-back
            # matmuls on scratch data into a dedicated scratch PSUM bank
            # (never read). Real matmuls then start un-throttled.
            scratch = consts.tile([128, 256], COMPUTE_DT)
            nc.gpsimd.memset(scratch[:], 0.0)
            wpsum = warm_pool.tile([128, 512], mybir.dt.float32)
            for _ in range(20):
                nc.tensor.matmul(wpsum[:, 0:256], scratch[:, 0:128],
                                 scratch[:], start=True, stop=True)

            def rep_body():
                for g in range(GROUPS):
                    xt = xpool.tile([128, MT, GROUP_COLS], COMPUTE_DT,
                                    name="xt", tag="xt")
                    # input prefetch on ACT HWDGE ring, separate from the
                    # output DMAs on the SP ring
                    nc.scalar.dma_start(xt[:], xt_d[g])
                    for half in range(CHUNKS_PER_GROUP):
                        r = g * CHUNKS_PER_GROUP + half
                        cs = slice(half * 128, (half + 1) * 128)
                        chunk_body(r, lambda m, cs=cs: xt[:, m, cs],
                                   r == CHUNKS - 1)

            if reps == 1:
                rep_body()
            else:
                assert reps % REPS_UNROLL == 0
                with tc.For_i(0, reps // REPS_UNROLL):
                    for _ in range(REPS_UNROLL):
                        rep_body()
    nc.compile()
    return nc


_NC_CACHE = {}


def _get_nc(reps=1):
    if reps not in _NC_CACHE:
        _NC_CACHE[reps] = _build_nc(reps)
    return _NC_CACHE[reps]


def _prep_core_input(xb):
    """xb: [C, H, W] float32 -> device input (bf16).

    xt[g, p, m, c] = X[128m+p, 256g+c] where X = x.T zero-padded by 64."""
    rows = np.ascontiguousarray(xb.reshape(ROWS, W))
    X = np.zeros((XLEN, ROWS), dtype=COMPUTE_NP)
    X[64:64 + W, :] = rows.T.astype(COMPUTE_NP)
    xt = X.reshape(MT, 128, GROUPS, GROUP_COLS)
    return {"xt": np.ascontiguousarray(xt.transpose(2, 1, 0, 3))}


def _in_maps(x):
    wt = _weights()
    return [dict(_prep_core_input(x[b]), wt=wt) for b in range(N_CORES)]


def _main_perm():
    """perm[s] maps decimated-q index -> column within the 352-wide block."""
    col_of = {sq: c for c, sq in enumerate(_BCOLS + _SCOLS + _ACOLS)}
    return [np.array([col_of[(s, q)] for q in range(0, 128, DEC[s])])
            for s in range(4)]


def _fft_upsample(yd, m):
    """Exact band-limited upsample: yd [..., 1152//m] covering positions
    [-64, 1088) step m -> full-rate [..., 1024] at positions [0, 1024)."""
    nb = XLEN // (2 * m)  # live rfft bins of the decimated signal
    F = np.fft.rfft(yd, axis=-1)
    Ff = np.zeros(F.shape[:-1] + (XLEN // 2 + 1,), np.complex128)
    Ff[..., :nb] = m * F[..., :nb]
    return np.fft.irfft(Ff, n=XLEN, axis=-1)[..., 64:64 + W].astype(np.float32)


def _postprocess(out_dev):
    """out_dev: [16, 128, 2912] bf16 (permuted cols) -> [C, S, H, W] f32."""
    o = np.asarray(out_dev).astype(np.float32)
    main = o[..., :JT * MW].reshape(C, 128, JT, MW)
    perm = _main_perm()
    y2 = main[..., perm[0]].reshape(C, 128, W)
    y4 = main[..., perm[1]].reshape(C, 128, W)
    el, er = o[..., JT * MW:JT * MW + EW], o[..., JT * MW + EW:]
    yd8 = np.concatenate(
        [el[..., 0:32], main[..., perm[2]].reshape(C, 128, W // 2),
         er[..., 0:32]], axis=-1)
    yd16 = np.concatenate(
        [el[..., 32:48], main[..., perm[3]].reshape(C, 128, W // 4),
         er[..., 32:48]], axis=-1)
    y8 = _fft_upsample(yd8, 2)
    y16 = _fft_upsample(yd16, 4)
    return np.stack([y2, y4, y8, y16], axis=1)  # [C, S, H, W]


def kernel(x):
    x = np.asarray(x, dtype=np.float32)
    assert x.shape == (B, C, H, W)
    in_maps = _in_maps(x)
    nc = _get_nc()
    res = run_bass_kernel_spmd(nc, in_maps, core_ids=list(range(N_CORES)))
    out = np.stack([_postprocess(res.results[b]["out"]) for b in range(N_CORES)])
    return out  # [B, C, S, H, W] float32
